# revision 6
# baseline (speedup 1.0000x reference)
"""Trainium2 Bass kernel for nn_NeuralPredictor (GNN message passing).

Strategy (matches sharding hint): shard the 18000-edge dimension across the
8 NeuronCores (2250 edges each, zero-padded to 2304 = 18*128). Each core:
  1. bvm_meta = b_variable_mask @ meta_data           (replicated, tiny)
  2. graph_featT[32, e] via bvm_meta.T @ vmask (natural layout, K=vars)
  3. edge-MLP (feature-major L1, edge-major L2) for both branches -> h2
  4. mask^T @ h2 partial node aggregates (stationary bf16 mask tiles)
  5. ReduceScatter(add) over the 8 cores -> each core owns a node slice
  6. node-MLP + sigmoid on the local slice; host concatenates slices.

All PE matmuls run in bf16 (masks are 0/1 -> exact in bf16) with fp32 PSUM
accumulation. Biases are folded into the matmuls as extra contraction rows
paired with a constant ones row.
"""

import numpy as np
import ml_dtypes

import concourse.bass as bass
import concourse.mybir as mybir
import concourse.tile as tile
from concourse import bacc
from concourse.bass_utils import run_bass_kernel_spmd
from concourse.masks import make_identity

BF16 = mybir.dt.bfloat16
F32 = mybir.dt.float32
NPBF16 = ml_dtypes.bfloat16

NCORES = 8
E, NV, NF, B = 18000, 1500, 6000, 16
DEC, META, H = 100, 32, 150          # H = MEM_H = MEM_AGG_H = AGG_H
ESH = E // NCORES                    # 2250 true edges per core
EPAD = 2304                          # 18 * 128
NKE = EPAD // 128                    # 18 edge k-tiles
VOUT = 1504                          # padded var nodes (8 * 188)
VLOC = VOUT // NCORES                # 188
VK = 1536                            # var contraction pad (12 * 128)
NKV = VK // 128                      # 12
FOUT = 6016                          # padded fn nodes (8 * 752 = 4 * 1504)
FLOC = FOUT // NCORES                # 752
NG = 1504                            # node group size for f aggregation
NFG = FOUT // NG                     # 4 groups
GLOC = NG // NCORES                  # 188 rows per core per group

# edge blocks within the 2304-wide shard
EBLOCKS = [(0, 512), (512, 512), (1024, 512), (1536, 512), (2048, 256)]
# node tiles within a 1504-wide group
def _ntiles(total):
    out = []
    o = 0
    while o < total:
        out.append((o, min(128, total - o)))
        o += 128
    return out

NTILES_G = _ntiles(NG)               # 11x128 + 96

_CACHED = {}


def _build():
    if "nc" in _CACHED:
        return _CACHED["nc"]
    nc = bacc.Bacc("TRN2", target_bir_lowering=False, debug=False,
                   num_devices=NCORES)

    # ---- DRAM I/O ----
    d_sv = nc.dram_tensor("sv_in", [101, EPAD], BF16, kind="ExternalInput")
    d_sf = nc.dram_tensor("sf_in", [101, EPAD], BF16, kind="ExternalInput")
    d_vmn = nc.dram_tensor("vmask_n", [VK, EPAD], BF16, kind="ExternalInput")
    d_vmt = nc.dram_tensor("vmask_t", [EPAD, VOUT], BF16, kind="ExternalInput")
    d_fmt = nc.dram_tensor("fmask_t", [EPAD, FOUT], BF16, kind="ExternalInput")
    d_bvmt = nc.dram_tensor("bvm_t", [16, VK], BF16, kind="ExternalInput")
    d_meta = nc.dram_tensor("meta_in", [16, META], BF16, kind="ExternalInput")
    dw = {}
    for p in ("v", "f"):
        for nm, shp in (("w1a", [101, H]), ("w1b", [32, H]), ("w1c", [1, H]),
                        ("w2a", [128, H]), ("w2b", [22, H]), ("w2c", [1, H]),
                        ("wa1a", [128, H]), ("wa1b", [22, H]),
                        ("wa1c", [1, H]),
                        ("wa2a", [128, DEC]), ("wa2b", [22, DEC]),
                        ("wa2c", [1, DEC]),
                        ("wc", [DEC, 1]), ("wcb", [1, 1])):
            dw[p + nm] = nc.dram_tensor(p + nm, shp, BF16,
                                        kind="ExternalInput")
    d_vp = nc.dram_tensor("v_pred", [1, VLOC], F32, kind="ExternalOutput")
    d_fp = nc.dram_tensor("f_pred", [1, FLOC], F32, kind="ExternalOutput")

    with tile.TileContext(nc) as tc:
        _device_program(nc, tc, d_sv, d_sf, d_vmn, d_vmt, d_fmt, d_bvmt,
                        d_meta, dw, d_vp, d_fp)
    nc.compile()
    _CACHED["nc"] = nc
    return nc


def _device_program(nc, tc, d_sv, d_sf, d_vmn, d_vmt, d_fmt, d_bvmt, d_meta,
                    dw, d_vp, d_fp):
    rg = [list(range(NCORES))]

    with tc.tile_pool(name="const", bufs=1) as cpool, \
         tc.tile_pool(name="dram", bufs=1, space="DRAM") as dpool:
        # ---- constants / weights resident in SBUF ----
        w = {k: cpool.tile_from(v.ap(), name=f"w_{k}") for k, v in dw.items()}
        bvm_sb = cpool.tile_from(d_bvmt.ap(), name="bvm_sb")
        meta_sb = cpool.tile_from(d_meta.ap(), name="meta_sb")
        ones = cpool.tile([1, 512], BF16)
        nc.gpsimd.memset(ones[:], 1.0)
        ident = cpool.tile([128, 128], F32)
        make_identity(nc, ident[:])

        svv = cpool.tile_from(d_sv.ap(), name="svv")   # [101, EPAD]
        svf = cpool.tile_from(d_sf.ap(), name="svf")
        h2v = cpool.tile([128, NKE * H], BF16)    # 18 tiles of [128, 150]
        h2f = cpool.tile([128, NKE * H], BF16)

        # DRAM bounce buffers for collectives
        v_agg_in = dpool.tile([VOUT, H], F32)
        v_agg_out = dpool.tile([VLOC, H], F32)
        f_agg_in = [dpool.tile([NG, H], F32, name=f"f_agg_in{g}")
                    for g in range(NFG)]
        f_agg_out = [dpool.tile([GLOC, H], F32, name=f"f_agg_out{g}")
                     for g in range(NFG)]

        # ---- phase 1: bvm_meta [VK, 32] bf16 (v-major) ----
        bm_all = cpool.tile([128, NKV * META], BF16)
        with tc.tile_pool(name="pbm", bufs=2, space="PSUM") as pbm:
            for kv in range(NKV):
                ps = pbm.tile([128, META], F32, name=f"psbm{kv}", tag="psbm")
                nc.tensor.matmul(ps[:], bvm_sb[:, kv * 128:(kv + 1) * 128],
                                 meta_sb[:], start=True, stop=True)
                nc.vector.tensor_copy(bm_all[:, kv * META:(kv + 1) * META],
                                      ps[:])

        # v-agg mask slabs: prefetch early (no deps)
        with tc.tile_pool(name="vtpool", bufs=1) as vtpool, \
             tc.tile_pool(name="vmnpool", bufs=1) as vmnpool:
            vt_all = vtpool.tile([128, NKE * VOUT], BF16)     # 6.9 MB
            for k in range(NKE):
                nc.sync.dma_start(vt_all[:, k * VOUT:(k + 1) * VOUT],
                                  d_vmt[k * 128:(k + 1) * 128, :])
            vmn_all = vmnpool.tile([128, NKV * EPAD], BF16)   # 7.1 MB
            for kv in range(NKV):
                nc.sync.dma_start(vmn_all[:, kv * EPAD:(kv + 1) * EPAD],
                                  d_vmn[kv * 128:(kv + 1) * 128, :])

            # ---- phase 2: graph features + edge MLP ----
            with tc.tile_pool(name="ph2pools", bufs=1) as spool, \
                 tc.tile_pool(name="pgf", bufs=2, space="PSUM") as pgf, \
                 tc.tile_pool(name="ph1a", bufs=2, space="PSUM") as ph1a, \
                 tc.tile_pool(name="ph1b", bufs=2, space="PSUM") as ph1b, \
                 tc.tile_pool(name="ph2", bufs=2, space="PSUM") as ph2:
                for bi, (e0, bw) in enumerate(EBLOCKS):
                    # graph_featT [32, bw] for this block
                    psg = pgf.tile([32, 512], F32, name=f"psg{bi}", tag="psg")
                    for kv in range(NKV):
                        nc.tensor.matmul(
                            psg[:, :bw],
                            bm_all[:, kv * META:(kv + 1) * META],
                            vmn_all[:, kv * EPAD + e0:kv * EPAD + e0 + bw],
                            start=(kv == 0), stop=(kv == NKV - 1))
                    gft = spool.tile([32, 512], BF16, name=f"gft{bi}",
                                     tag="gft", bufs=2)
                    nc.vector.tensor_copy(gft[:, :bw], psg[:, :bw])

                    for br, (sv_sb, h2_sb) in enumerate(
                            ((svv, h2v), (svf, h2f))):
                        p = "vf"[br]
                        # L1 feature-major: h1T[150, bw]
                        h1a = spool.tile([128, 512], BF16,
                                         name=f"h1a{bi}{p}", tag="h1a", bufs=3)
                        h1b = spool.tile([22, 512], BF16,
                                         name=f"h1b{bi}{p}", tag="h1b", bufs=3)
                        for mh, (m0, mw) in enumerate(((0, 128), (128, 22))):
                            psh = (ph1a if mh == 0 else ph1b).tile(
                                [mw, 512], F32, name=f"psh{bi}{p}{mh}",
                                tag=f"psh{mh}")
                            nc.tensor.matmul(psh[:, :bw],
                                             w[p + "w1a"][:, m0:m0 + mw],
                                             sv_sb[:, e0:e0 + bw],
                                             start=True, stop=False)
                            nc.tensor.matmul(psh[:, :bw],
                                             w[p + "w1b"][:, m0:m0 + mw],
                                             gft[:, :bw],
                                             start=False, stop=False)
                            nc.tensor.matmul(psh[:, :bw],
                                             w[p + "w1c"][:, m0:m0 + mw],
                                             ones[:, :bw],
                                             start=False, stop=True)
                            dst = h1a if mh == 0 else h1b
                            nc.scalar.activation(
                                dst[:mw, :bw], psh[:, :bw],
                                mybir.ActivationFunctionType.Relu)
                        # L2 edge-major: h2[128e, 150] per edge tile
                        for t in range(bw // 128):
                            k = (e0 + t * 128) // 128
                            c0 = t * 128
                            psq = ph2.tile([128, H], F32,
                                           name=f"psq{bi}{p}{t}", tag="psq")
                            nc.tensor.matmul(psq[:], h1a[:, c0:c0 + 128],
                                             w[p + "w2a"][:],
                                             start=True, stop=False)
                            nc.tensor.matmul(psq[:], h1b[:, c0:c0 + 128],
                                             w[p + "w2b"][:],
                                             start=False, stop=False)
                            nc.tensor.matmul(psq[:], ones[:, :128],
                                             w[p + "w2c"][:],
                                             start=False, stop=True)
                            nc.scalar.activation(
                                h2_sb[:, k * H:(k + 1) * H], psq[:],
                                mybir.ActivationFunctionType.Relu)

            # ---- phase 3: v aggregation + ReduceScatter ----
            with tc.tile_pool(name="paggv", bufs=4, space="PSUM") as pagg, \
                 tc.tile_pool(name="stgv", bufs=4) as stg:
                for mi, (m0, mw) in enumerate(_ntiles(VOUT)):
                    psa = pagg.tile([128, H], F32, name=f"psav{mi}",
                                    tag="psa")
                    for k in range(NKE):
                        nc.tensor.matmul(
                            psa[:mw, :],
                            vt_all[:, k * VOUT + m0:k * VOUT + m0 + mw],
                            h2v[:, k * H:(k + 1) * H],
                            start=(k == 0), stop=(k == NKE - 1))
                    st = stg.tile([128, H], F32, name=f"stv{mi}", tag="stv")
                    nc.vector.tensor_copy(st[:mw, :], psa[:mw, :])
                    nc.sync.dma_start(v_agg_in[m0:m0 + mw, :], st[:mw, :])
                nc.gpsimd.collective_compute(
                    "ReduceScatter", mybir.AluOpType.add, replica_groups=rg,
                    ins=[v_agg_in[:].opt()], outs=[v_agg_out[:].opt()])

        # ---- phase 4: f aggregation per group + ReduceScatter ----
        with tc.tile_pool(name="ftpool", bufs=24) as ftpool, \
             tc.tile_pool(name="paggf", bufs=4, space="PSUM") as pagg, \
             tc.tile_pool(name="stgf", bufs=4) as stg:
            for g in range(NFG):
                fslabs = []
                for k in range(NKE):
                    sl = ftpool.tile([128, NG], BF16, name=f"ft{g}_{k}",
                                     tag="ft")
                    nc.sync.dma_start(
                        sl[:], d_fmt[k * 128:(k + 1) * 128,
                                     g * NG:(g + 1) * NG])
                    fslabs.append(sl)
                for mi, (m0, mw) in enumerate(NTILES_G):
                    psa = pagg.tile([128, H], F32, name=f"psaf{g}_{mi}",
                                    tag="psa")
                    for k in range(NKE):
                        nc.tensor.matmul(
                            psa[:mw, :], fslabs[k][:, m0:m0 + mw],
                            h2f[:, k * H:(k + 1) * H],
                            start=(k == 0), stop=(k == NKE - 1))
                    st = stg.tile([128, H], F32, name=f"stf{g}_{mi}",
                                  tag="stf")
                    nc.vector.tensor_copy(st[:mw, :], psa[:mw, :])
                    nc.sync.dma_start(f_agg_in[g][m0:m0 + mw, :],
                                      st[:mw, :])
                nc.gpsimd.collective_compute(
                    "ReduceScatter", mybir.AluOpType.add, replica_groups=rg,
                    ins=[f_agg_in[g][:].opt()], outs=[f_agg_out[g][:].opt()])

        # ---- phase 5: node MLP on local slices ----
        with tc.tile_pool(name="node", bufs=2) as npool, \
             tc.tile_pool(name="ptr", bufs=2, space="PSUM") as ptr, \
             tc.tile_pool(name="pnm", bufs=1, space="PSUM") as pnm:
            passes = [("v", v_agg_out, d_vp, 0)]
            for g in range(NFG):
                passes.append(("f", f_agg_out[g], d_fp, g * GLOC))
            for pi, (p, src, dst, oc0) in enumerate(passes):
                n = VLOC if p == "v" else GLOC  # both 188
                _node_mlp(nc, npool, ptr, pnm, w, ones, ident, src, dst,
                          oc0, n, p, pi)


def _node_mlp(nc, npool, ptr, pnm, w, ones, ident, src, dst, oc0, n, p, pi):
    """Node-side MLP for one local slice: src [n,150] f32 -> dst[0, oc0:oc0+n]."""
    # load node-major rows and transpose to feature-major aggT
    agga = npool.tile([128, 256], BF16, name=f"agga{pi}", tag="agga")
    aggb = npool.tile([22, 256], BF16, name=f"aggb{pi}", tag="aggb")
    for ni, (n0, nw) in enumerate(_ntiles(n)):
        raw = npool.tile([128, H], F32, name=f"raw{pi}_{ni}", tag="raw")
        nc.sync.dma_start(raw[:nw, :], src[n0:n0 + nw, :])
        for fi, (f0, fw) in enumerate(((0, 128), (128, 22))):
            pst = ptr.tile([128, 128], F32, name=f"pst{pi}_{ni}_{fi}",
                           tag="pst")
            nc.tensor.matmul(pst[:fw, :nw], raw[:nw, f0:f0 + fw],
                             ident[:nw, :nw], is_transpose=True,
                             start=True, stop=True)
            dst_t = agga if fi == 0 else aggb
            nc.vector.tensor_copy(dst_t[:fw, n0:n0 + nw], pst[:fw, :nw])
    # L1: aT[150, n] = relu(W1a_ext.T @ aggT_ext)
    a1a = npool.tile([128, 256], BF16, name=f"a1a{pi}", tag="a1a")
    a1b = npool.tile([22, 256], BF16, name=f"a1b{pi}", tag="a1b")
    for mh, (m0, mw) in enumerate(((0, 128), (128, 22))):
        ps = pnm.tile([mw, 256], F32, name=f"psn1{pi}_{mh}", tag=f"psn1{mh}")
        nc.tensor.matmul(ps[:, :n], w[p + "wa1a"][:, m0:m0 + mw],
                         agga[:, :n], start=True, stop=False)
        nc.tensor.matmul(ps[:, :n], w[p + "wa1b"][:, m0:m0 + mw],
                         aggb[:, :n], start=False, stop=False)
        nc.tensor.matmul(ps[:, :n], w[p + "wa1c"][:, m0:m0 + mw],
                         ones[:, :n], start=False, stop=True)
        dst_t = a1a if mh == 0 else a1b
        nc.scalar.activation(dst_t[:mw, :n], ps[:, :n],
                             mybir.ActivationFunctionType.Relu)

    # L2: outT[100, n] (no relu)
    ot = npool.tile([DEC, 256], BF16, name=f"ot{pi}", tag="ot")
    ps2 = pnm.tile([DEC, 256], F32, name=f"psn2{pi}", tag="psn2")
    nc.tensor.matmul(ps2[:, :n], w[p + "wa2a"][:], a1a[:, :n],
                     start=True, stop=False)
    nc.tensor.matmul(ps2[:, :n], w[p + "wa2b"][:], a1b[:, :n],
                     start=False, stop=False)
    nc.tensor.matmul(ps2[:, :n], w[p + "wa2c"][:], ones[:, :n],
                     start=False, stop=True)
    nc.vector.tensor_copy(ot[:, :n], ps2[:, :n])

    # classifier + sigmoid
    psz = pnm.tile([1, 256], F32, name=f"psz{pi}", tag="psz")
    nc.tensor.matmul(psz[:, :n], w[p + "wc"][:], ot[:, :n],
                     start=True, stop=False)
    nc.tensor.matmul(psz[:, :n], w[p + "wcb"][:], ones[:, :n],
                     start=False, stop=True)
    pred = npool.tile([1, 256], F32, name=f"pred{pi}", tag="pred")
    nc.scalar.activation(pred[:, :n], psz[:, :n],
                         mybir.ActivationFunctionType.Sigmoid)
    nc.sync.dma_start(dst[:, oc0:oc0 + n], pred[:, :n])


# ---------------- host side ----------------

def _prep_in_maps(inputs):
    f32 = np.float32
    g = {k: np.asarray(v, dtype=f32) for k, v in inputs.items()}
    dvs, dfs = g["decimator_variable_state"], g["decimator_function_state"]
    ef, meta = g["edge_feature"], g["meta_data"]
    vm, fm, bvm = g["variable_mask"], g["function_mask"], g["b_variable_mask"]

    shared = {}
    bvm_t = np.zeros((16, VK), NPBF16)
    bvm_t[:, :NV] = bvm.T
    shared["bvm_t"] = bvm_t
    shared["meta_in"] = meta.astype(NPBF16)
    for p in ("v", "f"):
        W1m, b1m = g[p + "W1m"], g[p + "b1m"]
        W2m, b2m = g[p + "W2m"], g[p + "b2m"]
        W1a, b1a = g[p + "W1a"], g[p + "b1a"]
        W2a, b2a = g[p + "W2a"], g[p + "b2a"]
        Wc, bc = g[p + "Wc"], g[p + "bc"]
        shared[p + "w1a"] = W1m[0:101].astype(NPBF16)
        shared[p + "w1b"] = W1m[101:133].astype(NPBF16)
        shared[p + "w1c"] = b1m[None, :].astype(NPBF16)
        shared[p + "w2a"] = W2m[0:128].astype(NPBF16)
        shared[p + "w2b"] = W2m[128:150].astype(NPBF16)
        shared[p + "w2c"] = b2m[None, :].astype(NPBF16)
        shared[p + "wa1a"] = W1a[0:128].astype(NPBF16)
        shared[p + "wa1b"] = W1a[128:150].astype(NPBF16)
        shared[p + "wa1c"] = b1a[None, :].astype(NPBF16)
        shared[p + "wa2a"] = W2a[0:128].astype(NPBF16)
        shared[p + "wa2b"] = W2a[128:150].astype(NPBF16)
        shared[p + "wa2c"] = b2a[None, :].astype(NPBF16)
        shared[p + "wc"] = Wc.astype(NPBF16)
        shared[p + "wcb"] = bc[None, :].astype(NPBF16)

    in_maps = []
    for c in range(NCORES):
        sl = slice(c * ESH, (c + 1) * ESH)
        m = dict(shared)
        sv = np.zeros((101, EPAD), NPBF16)
        sv[0:100, :ESH] = dvs[sl].T
        sv[100, :ESH] = ef[sl, 0]
        m["sv_in"] = sv
        sf = np.zeros((101, EPAD), NPBF16)
        sf[0:100, :ESH] = dfs[sl].T
        sf[100, :ESH] = ef[sl, 0]
        m["sf_in"] = sf
        vmn = np.zeros((VK, EPAD), NPBF16)
        vmn[:NV, :ESH] = vm[:, sl]
        m["vmask_n"] = vmn
        vmt = np.zeros((EPAD, VOUT), NPBF16)
        vmt[:ESH, :NV] = vm[:, sl].T
        m["vmask_t"] = vmt
        fmt = np.zeros((EPAD, FOUT), NPBF16)
        fmt[:ESH, :NF] = fm[:, sl].T
        m["fmask_t"] = fmt
        in_maps.append(m)
    return in_maps


def _assemble(results):
    vp = np.concatenate([results[c]["v_pred"][0] for c in range(NCORES)])
    fp = np.empty(FOUT, np.float32)
    for c in range(NCORES):
        fpc = results[c]["f_pred"][0]
        for gi in range(NFG):
            fp[gi * NG + c * GLOC: gi * NG + (c + 1) * GLOC] = \
                fpc[gi * GLOC:(gi + 1) * GLOC]
    return (vp[:NV, None].astype(np.float32), fp[:NF, None].astype(np.float32))


def kernel(**inputs):
    nc = _build()
    in_maps = _prep_in_maps(inputs)
    res = run_bass_kernel_spmd(nc, in_maps, core_ids=list(range(NCORES)))
    return _assemble(res.results)
